# revision 27
# baseline (speedup 1.0000x reference)
"""Trainium2 Bass kernel for nn_MultiHeadCrossAttention (B=32, Nc=2048, H=8, topk=12).

kernel(**inputs) takes FULL inputs, returns FULL output [32, 1, 128].
Batch is sharded 4-per-core across 8 NeuronCores (data parallel, no collectives).

Per-batch device algorithm (rows=(h,q) 128 wide, j = e*2048+nc in [0,16384)):
  hoisted for all 4 batches: qbd (block-diag 0.25-scaled Q), A_e fp16
  S_chunk[row, 1024] = Ah.T @ Ch single fp16 term, directly in PSUM
  per-chunk top8 (DVE max8) + max_index read straight from PSUM
  VT_e[hd,nc] = Wv_e.T @ Ch -> VT [128,16384] fp32 (ScE 1024-wide copies)
  exact global top-12 marking via max8/match_replace rounds on cand
  pack (global_idx*1024 + quantized_value), extract winners via max8
  weights = exp(value)/sum
  G = ap_gather(VT, winner idx, d=1 fp32)
  PV^T[(h,d),q] = sum w*G  (headrep matmul broadcasts weights)
  out = (PV flat @ WjwP) + x;  out = out @ Wp + bp
"""

import sys
import numpy as np

for p in ("/opt/trn_rl_repo",):
    if p not in sys.path:
        sys.path.insert(0, p)

import ml_dtypes

B, CORES, BPC = 32, 8, 4
H, HD, NQ, TK, C, NC = 8, 16, 16, 12, 128, 2048
NJ = 8 * NC            # 16384
CHUNK = 1024
NCH = NJ // CHUNK      # 16
NCAND = NCH * 8        # 128
NEG = -1e30
MAGIC = 12582912.0     # 2**23 + 2**22: add/sub rounds fp32 to nearest int

_prog_cache = {}


def _build_program():
    import concourse.bass as bass
    import concourse.mybir as mybir
    import concourse.tile as tile
    from concourse import bacc
    from concourse import library_config

    dt = mybir.dt
    Alu = mybir.AluOpType
    f32, f16, bf16 = dt.float32, dt.float16, dt.bfloat16
    nc = bacc.Bacc("TRN2", target_bir_lowering=False)

    comphT_d = nc.dram_tensor("comphT", [BPC, C, NC], f16, kind="ExternalInput")
    a16h4_d = nc.dram_tensor("a16h4", [BPC, C, 8 * C], f16, kind="ExternalInput")
    wv_d = nc.dram_tensor("wv", [C, 8 * C], f16, kind="ExternalInput")
    wjwp_d = nc.dram_tensor("wjwp", [C, NQ * C], f32, kind="ExternalInput")
    xp4_d = nc.dram_tensor("xp4", [BPC, C], f32, kind="ExternalInput")
    hrep_d = nc.dram_tensor("hrep", [C, C], f32, kind="ExternalInput")
    choff_d = nc.dram_tensor("choff", [C, NCAND], f32, kind="ExternalInput")
    m192_d = nc.dram_tensor("m192", [C, 192], f32, kind="ExternalInput")
    out_d = nc.dram_tensor("out", [BPC, C], f32, kind="ExternalOutput")

    with tile.TileContext(nc) as tc:
        nc.gpsimd.load_library(library_config.ap_gather)
        with (
            tc.tile_pool(name="weights", bufs=1) as wpool,
            tc.tile_pool(name="compt", bufs=3) as ctpool,
            tc.tile_pool(name="bigV", bufs=1) as vpool,
            tc.tile_pool(name="small", bufs=3) as smpool,
            tc.tile_pool(name="ps_s", bufs=2, space="PSUM") as ps_s,
            tc.tile_pool(name="ps_w", bufs=2, space="PSUM") as ps_w,
        ):
            # ---- critical-path DMAs first: A for b0, comp for b0 ----
            a16h_l = []
            for b in range(BPC):
                t = wpool.tile([C, 8 * C], f16, tag=f"a16h{b}")
                a16h_l.append(t)
            nc.sync.dma_start(a16h_l[0][:], a16h4_d[0])
            c16h_0 = ctpool.tile([C, NC], f16, tag="c16h")
            nc.scalar.dma_start(c16h_0[:], comphT_d[0])
            for b in range(1, BPC):
                nc.sync.dma_start(a16h_l[b][:], a16h4_d[b])
            wv_s = wpool.tile([C, 8 * C], f16)
            nc.sync.dma_start(wv_s[:], wv_d[:])
            wjwp_s = wpool.tile([C, NQ * C], f32)
            nc.sync.dma_start(wjwp_s[:], wjwp_d[:])
            xp4_s = wpool.tile([BPC, C], f32)
            nc.sync.dma_start(xp4_s[:], xp4_d[:])
            hrep_s = wpool.tile([C, C], f32)
            nc.sync.dma_start(hrep_s[:], hrep_d[:])
            choff_s = wpool.tile([C, NCAND], f32)
            nc.sync.dma_start(choff_s[:], choff_d[:])
            m192_s = wpool.tile([C, 192], f32)
            nc.sync.dma_start(m192_s[:], m192_d[:])

            pvt4_s = wpool.tile([C, NQ * BPC], f32)   # [(h,d), (q,b)]

            for b in range(BPC):
                a16h = a16h_l[b]
                if b == 0:
                    c16h = c16h_0
                else:
                    c16h = ctpool.tile([C, NC], f16, tag="c16h")
                    nc.sync.dma_start(c16h[:], comphT_d[b])

                # ---- S chunks in PSUM: single fp16 term; scan from PSUM ----
                cand_s = smpool.tile([C, NCAND], f32, tag="cand")
                li_s = smpool.tile([C, NCAND], dt.uint16, tag="li")
                for ch in range(NCH):
                    e, half = ch // 2, ch % 2
                    ah = a16h[:, e * C:(e + 1) * C]
                    s_ps = ps_s.tile([C, CHUNK], f32, tag="s")
                    for n in range(2):
                        col = half * 1024 + n * 512
                        nc.tensor.matmul(
                            s_ps[:, n * 512:(n + 1) * 512],
                            ah, c16h[:, col:col + 512],
                        )
                    nc.vector.max(cand_s[:, ch * 8:(ch + 1) * 8], s_ps[:])
                    nc.vector.max_index(
                        li_s[:, ch * 8:(ch + 1) * 8],
                        cand_s[:, ch * 8:(ch + 1) * 8],
                        s_ps[:],
                    )

                # ---- lif convert on ScE (uint16 -> f32), before V copies ----
                lif = smpool.tile([C, NCAND], f32, tag="lif")
                nc.scalar.copy(lif[:], li_s[:])

                # ---- V^T fp32 (PE after S; ScE 1024-wide copies) ----
                vt_s = vpool.tile([C, NJ], f32, tag="VT")
                for e in range(8):
                    for half in range(2):
                        v_ps = ps_w.tile([C, 1024], f32, tag="w")
                        for n in range(2):
                            col = half * 1024 + n * 512
                            nc.tensor.matmul(
                                v_ps[:, n * 512:(n + 1) * 512],
                                wv_s[:, e * C:(e + 1) * C],
                                c16h[:, col:col + 512],
                            )
                        nc.scalar.copy(
                            vt_s[:, e * NC + half * 1024: e * NC + (half + 1) * 1024],
                            v_ps[:],
                        )

                # ---- exact top-12 marking on cand ----
                t8a = smpool.tile([C, 8], f32, tag="t8a")
                nc.vector.max(t8a[:], cand_s[:])
                c2 = smpool.tile([C, NCAND], f32, tag="c2")
                nc.vector.match_replace(c2[:], t8a[:], cand_s[:], NEG)
                t8b = smpool.tile([C, 8], f32, tag="t8b")
                nc.vector.max(t8b[:], c2[:])
                nc.vector.tensor_scalar(
                    t8b[:, 4:8], t8b[:, 4:8], 0.0, 1e30, Alu.mult, Alu.add
                )
                rr = smpool.tile([C, NCAND], f32, tag="rr")
                nc.vector.match_replace(rr[:], t8b[:], c2[:], NEG)

                # ---- pack global_idx*1024 + q10(value); mask; extract ----
                gfl = smpool.tile([C, NCAND], f32, tag="gfl")
                nc.vector.scalar_tensor_tensor(
                    gfl[:], lif[:], 1024.0, choff_s[:], Alu.mult, Alu.add
                )
                q10 = smpool.tile([C, NCAND], f32, tag="q10")
                nc.vector.tensor_scalar(
                    q10[:], cand_s[:], 4.0, 128.0, Alu.add, Alu.mult
                )
                pmsum = smpool.tile([C, NCAND], f32, tag="pmsum")
                nc.vector.scalar_tensor_tensor(
                    pmsum[:], gfl[:], 1.0, q10[:], Alu.mult, Alu.add
                )
                pm = smpool.tile([C, NCAND], f32, tag="pm")
                nc.vector.scalar_tensor_tensor(
                    pm[:], rr[:], -1e29, pmsum[:], Alu.is_le, Alu.mult
                )

                pw = smpool.tile([C, 16], f32, tag="pw")
                nc.vector.max(pw[:, 0:8], pm[:])
                pm2 = smpool.tile([C, NCAND], f32, tag="pm2")
                nc.vector.match_replace(pm2[:], pw[:, 0:8], pm[:], 0.0)
                nc.vector.max(pw[:, 8:16], pm2[:])

                # ---- decode winners: gidx + value -> weights ----
                gidxf = smpool.tile([C, 16], f32, tag="gidxf")
                nc.vector.tensor_scalar(
                    gidxf[:], pw[:], 1.0 / 1024.0, -0.5, Alu.mult, Alu.add
                )
                nc.vector.tensor_scalar(
                    gidxf[:], gidxf[:], MAGIC, MAGIC, Alu.add, Alu.subtract
                )
                # winner index first so the gather starts ASAP
                gp_i = smpool.tile([C, TK], dt.int16, tag="gpi")
                nc.vector.tensor_copy(gp_i[:], gidxf[:, 0:TK])
                g_s = smpool.tile([C, TK * NQ], f32, tag="G")
                nc.gpsimd.ap_gather(
                    g_s[:], vt_s[:], gp_i[:],
                    channels=C, num_elems=NJ, d=1, num_idxs=TK * NQ,
                )

                vv = smpool.tile([C, 16], f32, tag="vv")
                nc.vector.scalar_tensor_tensor(
                    vv[:], gidxf[:], -1024.0, pw[:], Alu.mult, Alu.add
                )
                nc.vector.tensor_scalar(
                    vv[:], vv[:], 1.0 / 128.0, -4.0, Alu.mult, Alu.add
                )
                expv = smpool.tile([C, 16], f32, tag="expv")
                nc.scalar.activation(
                    expv[:], vv[:], mybir.ActivationFunctionType.Exp
                )
                wgt = smpool.tile([C, 16], f32, tag="wgt")
                den = smpool.tile([C, 1], f32, tag="den")
                nc.vector.scalar_tensor_tensor(
                    wgt[:], pw[:], 0.5, expv[:], Alu.is_ge, Alu.mult,
                    accum_out=den[:],
                )
                rden = smpool.tile([C, 1], f32, tag="rden")
                nc.vector.reciprocal(rden[:], den[:])
                wn = smpool.tile([C, 16], f32, tag="wn")
                nc.vector.tensor_scalar(wn[:], wgt[:], rden[:], None, Alu.mult)

                # ---- weights -> [(h,d), (i,q)] via headrep matmul ----
                wnb = (
                    wn[:, 0:TK].rearrange("p (i o) -> p i o", o=1)
                    .to_broadcast([C, TK, 16])
                )
                wsc = smpool.tile([C, TK * NQ], f32, tag="wsc")
                nc.vector.tensor_mul(
                    wsc[:].rearrange("p (i s) -> p i s", s=16),
                    wnb,
                    m192_s[:].rearrange("p (i s) -> p i s", s=16),
                )
                wb_ps = ps_w.tile([C, 1024], f32, tag="w")
                nc.tensor.matmul(wb_ps[:, 0:TK * NQ], hrep_s[:], wsc[:])

                gw = smpool.tile([C, TK * NQ], f32, tag="gw")
                nc.vector.tensor_mul(gw[:], g_s[:], wb_ps[:, 0:TK * NQ])
                # reduce over i, keep q: write PV^T into (q,b) cols
                nc.vector.tensor_reduce(
                    pvt4_s[:, b::BPC],
                    gw[:].rearrange("p (i q) -> p q i", q=NQ),
                    mybir.AxisListType.X,
                    Alu.add,
                )

            # ---- final projection (Wp folded into Wjw on host) ----
            # out[b, c_out] = sum_q pvt4[:, (q,b)]^T @ wjwp[:, (q,c_out)]
            # 4 interleaved accumulation chains to hide LDWEIGHTS/accum latency
            o1_ps = ps_w.tile([C, 1024], f32, tag="w")
            for q in range(NQ):
                nc.tensor.matmul(
                    o1_ps[0:BPC, 0:C],
                    pvt4_s[:, q * BPC:(q + 1) * BPC],
                    wjwp_s[:, q * C:(q + 1) * C],
                    start=(q == 0),
                    stop=(q == NQ - 1),
                )
            o4_s = smpool.tile([BPC, C], f32, tag="o4")
            nc.vector.tensor_add(o4_s[:], o1_ps[0:BPC, 0:C], xp4_s[:])
            nc.sync.dma_start(out_d[:], o4_s[:])

    nc.compile()
    return nc


def _host_prep(inputs):
    x = np.asarray(inputs["x"], dtype=np.float32)              # [32, 1, 128]
    complement = np.asarray(inputs["complement"], np.float32)  # [32, 2047, 128]
    Wq = np.asarray(inputs["Wq"], np.float32)
    Wkv = np.asarray(inputs["Wkv"], np.float32)
    Wjw = np.asarray(inputs["Wjw"], np.float32)
    Wp = np.asarray(inputs["Wp"], np.float32)
    bp = np.asarray(inputs["bp"], np.float32)

    wv = np.empty((C, 8 * C), np.float32)
    for e in range(8):
        wv[:, e * C:(e + 1) * C] = Wkv[:, e * 256 + 128: e * 256 + 256]
    wv = wv.astype(np.float16)
    # host A: A[b][c, (e, h, q)] = 0.25 * sum_hd Wk_e[c, (h,hd)] q[b, q, h, hd]
    kproj = Wkv.reshape(C, 8, 2, H, HD)[:, :, 0]          # [c, e, h, hd]
    qtb = (x.reshape(B, C) @ Wq).reshape(B, NQ, H, HD)    # [b, q, h, hd]
    a_all = 0.25 * np.einsum('cehd,bqhd->bcehq', kproj, qtb, optimize=True)
    a_all = a_all.reshape(B, C, 8 * C).astype(np.float16)
    # fold Wp: out = pv @ (Wjw @ Wp) + (x @ Wp + bp)
    Wjw2 = (Wjw.astype(np.float64) @ Wp.astype(np.float64)).astype(np.float32)
    wjwp = (
        Wjw2.reshape(H, NQ, HD, C).transpose(1, 0, 2, 3).reshape(NQ, C, C)
        .transpose(1, 0, 2).reshape(C, NQ * C)
    )
    hrep = np.kron(np.eye(H, dtype=np.float32), np.ones((HD, HD), np.float32))
    choffrow = ((np.arange(NCAND) // 8) * (CHUNK * 1024)).astype(np.float32)
    choff = np.tile(choffrow.reshape(1, NCAND), (C, 1))
    # m192[p, (i, s16)] = [s16 == p % 16], i in [0, 12)
    s_idx = np.tile(np.arange(16).reshape(1, 1, 16), (C, TK, 1))
    p_idx = (np.arange(C) % NQ).reshape(C, 1, 1)
    m192 = (s_idx == p_idx).astype(np.float32).reshape(C, TK * NQ)

    shared = dict(
        wv=np.ascontiguousarray(wv),
        wjwp=np.ascontiguousarray(wjwp),
        hrep=np.ascontiguousarray(hrep),
        choff=np.ascontiguousarray(choff),
        m192=np.ascontiguousarray(m192),
    )

    in_maps = []
    for core in range(CORES):
        bs = range(core * BPC, (core + 1) * BPC)
        comp = np.stack(
            [
                np.concatenate([x[b].reshape(1, C), complement[b]], axis=0)
                for b in bs
            ]
        ).astype(np.float32)
        compT = comp.transpose(0, 2, 1)          # [BPC, C, NC]
        comphT = compT.astype(np.float16)
        xb = x[list(bs)].reshape(BPC, C)
        xp4 = np.ascontiguousarray((xb @ Wp + bp).astype(np.float32))
        m = dict(shared)
        m["comphT"] = np.ascontiguousarray(comphT)
        m["a16h4"] = np.ascontiguousarray(a_all[list(bs)])
        m["xp4"] = xp4
        in_maps.append(m)
    return in_maps


def kernel(**inputs):
    from concourse.bass_utils import run_bass_kernel_spmd

    if "prog" not in _prog_cache:
        _prog_cache["prog"] = _build_program()
    nc = _prog_cache["prog"]

    in_maps = _host_prep(inputs)
    res = run_bass_kernel_spmd(nc, in_maps, core_ids=list(range(CORES)))
    out = np.empty((B, 1, C), np.float32)
    for core in range(CORES):
        o = res.results[core]["out"]
        for i in range(BPC):
            out[core * BPC + i, 0, :] = o[i]
    return out


if __name__ == "__main__":
    d = np.load("/root/problem/inputs_cache.npz")
    inputs = {k: d[k] for k in d.files}
    got = kernel(**inputs)
    print("kernel output:", got.shape, got.dtype, np.abs(got).max())


# revision 28
# speedup vs baseline: 1.0190x; 1.0190x over previous
"""Trainium2 Bass kernel for nn_MultiHeadCrossAttention (B=32, Nc=2048, H=8, topk=12).

kernel(**inputs) takes FULL inputs, returns FULL output [32, 1, 128].
Batch is sharded 4-per-core across 8 NeuronCores (data parallel, no collectives).

Per-batch device algorithm (rows=(h,q) 128 wide, j = e*2048+nc in [0,16384)):
  hoisted for all 4 batches: qbd (block-diag 0.25-scaled Q), A_e fp16
  S_chunk[row, 1024] = Ah.T @ Ch single fp16 term, directly in PSUM
  per-chunk top8 (DVE max8) + max_index read straight from PSUM
  VT_e[hd,nc] = Wv_e.T @ Ch -> VT [128,16384] fp32 (ScE 1024-wide copies)
  exact global top-12 marking via max8/match_replace rounds on cand
  pack (global_idx*1024 + quantized_value), extract winners via max8
  weights = exp(value)/sum
  G = ap_gather(VT, winner idx, d=1 fp32)
  PV^T[(h,d),q] = sum w*G  (headrep matmul broadcasts weights)
  out = (PV flat @ WjwP) + x;  out = out @ Wp + bp
"""

import sys
import numpy as np

for p in ("/opt/trn_rl_repo",):
    if p not in sys.path:
        sys.path.insert(0, p)

import ml_dtypes

B, CORES, BPC = 32, 8, 4
H, HD, NQ, TK, C, NC = 8, 16, 16, 12, 128, 2048
NJ = 8 * NC            # 16384
CHUNK = 1024
NCH = NJ // CHUNK      # 16
NCAND = NCH * 8        # 128
NEG = -1e30
MAGIC = 12582912.0     # 2**23 + 2**22: add/sub rounds fp32 to nearest int

_prog_cache = {}


def _build_program():
    import concourse.bass as bass
    import concourse.mybir as mybir
    import concourse.tile as tile
    from concourse import bacc
    from concourse import library_config

    dt = mybir.dt
    Alu = mybir.AluOpType
    f32, f16, bf16 = dt.float32, dt.float16, dt.bfloat16
    nc = bacc.Bacc("TRN2", target_bir_lowering=False)

    comphT_d = nc.dram_tensor("comphT", [BPC, C, NC], f16, kind="ExternalInput")
    a16h4_d = nc.dram_tensor("a16h4", [BPC, C, 8 * C], f16, kind="ExternalInput")
    wv_d = nc.dram_tensor("wv", [C, 8 * C], f16, kind="ExternalInput")
    wjwp_d = nc.dram_tensor("wjwp", [C, NQ * C], f32, kind="ExternalInput")
    xp4_d = nc.dram_tensor("xp4", [BPC, C], f32, kind="ExternalInput")
    hrep_d = nc.dram_tensor("hrep", [C, C], f32, kind="ExternalInput")
    choff_d = nc.dram_tensor("choff", [C, NCAND], f32, kind="ExternalInput")
    m192_d = nc.dram_tensor("m192", [C, 192], f32, kind="ExternalInput")
    out_d = nc.dram_tensor("out", [BPC, C], f32, kind="ExternalOutput")

    with tile.TileContext(nc) as tc:
        nc.gpsimd.load_library(library_config.ap_gather)
        with (
            tc.tile_pool(name="weights", bufs=1) as wpool,
            tc.tile_pool(name="compt", bufs=3) as ctpool,
            tc.tile_pool(name="bigV", bufs=1) as vpool,
            tc.tile_pool(name="small", bufs=3) as smpool,
            tc.tile_pool(name="ps_s", bufs=2, space="PSUM") as ps_s,
            tc.tile_pool(name="ps_w", bufs=2, space="PSUM") as ps_w,
        ):
            # ---- critical-path DMAs first: A for b0, comp for b0 ----
            a16h_l = []
            for b in range(BPC):
                t = wpool.tile([C, 8 * C], f16, tag=f"a16h{b}")
                a16h_l.append(t)
            nc.sync.dma_start(a16h_l[0][:], a16h4_d[0])
            c16h_0 = ctpool.tile([C, NC], f16, tag="c16h")
            nc.scalar.dma_start(c16h_0[:], comphT_d[0])
            for b in range(1, BPC):
                nc.sync.dma_start(a16h_l[b][:], a16h4_d[b])
            wv_s = wpool.tile([C, 8 * C], f16)
            nc.sync.dma_start(wv_s[:], wv_d[:])
            wjwp_s = wpool.tile([C, NQ * C], f32)
            nc.sync.dma_start(wjwp_s[:], wjwp_d[:])
            xp4_s = wpool.tile([BPC, C], f32)
            nc.sync.dma_start(xp4_s[:], xp4_d[:])
            hrep_s = wpool.tile([C, C], f32)
            nc.sync.dma_start(hrep_s[:], hrep_d[:])
            choff_s = wpool.tile([C, NCAND], f32)
            nc.sync.dma_start(choff_s[:], choff_d[:])
            m192_s = wpool.tile([C, 192], f32)
            nc.sync.dma_start(m192_s[:], m192_d[:])

            pvt4_s = wpool.tile([C, NQ * BPC], f32)   # [(h,d), (q,b)]

            for b in range(BPC):
                a16h = a16h_l[b]
                if b == 0:
                    c16h = c16h_0
                else:
                    c16h = ctpool.tile([C, NC], f16, tag="c16h")
                    nc.sync.dma_start(c16h[:], comphT_d[b])

                # ---- S chunks in PSUM; scans software-pipelined so each
                # find_index consumes the PREVIOUS chunk (breaks RAW stalls) ----
                cand_s = smpool.tile([C, NCAND], f32, tag="cand")
                li_s = smpool.tile([C, NCAND], dt.uint16, tag="li")
                prev = None
                for ch in range(NCH):
                    e, half = ch // 2, ch % 2
                    ah = a16h[:, e * C:(e + 1) * C]
                    s_ps = ps_s.tile([C, CHUNK], f32, tag="s")
                    for n in range(2):
                        col = half * 1024 + n * 512
                        nc.tensor.matmul(
                            s_ps[:, n * 512:(n + 1) * 512],
                            ah, c16h[:, col:col + 512],
                        )
                    nc.vector.max(cand_s[:, ch * 8:(ch + 1) * 8], s_ps[:])
                    if prev is not None:
                        pch, pps = prev
                        nc.vector.max_index(
                            li_s[:, pch * 8:(pch + 1) * 8],
                            cand_s[:, pch * 8:(pch + 1) * 8],
                            pps[:],
                        )
                    prev = (ch, s_ps)
                pch, pps = prev
                nc.vector.max_index(
                    li_s[:, pch * 8:(pch + 1) * 8],
                    cand_s[:, pch * 8:(pch + 1) * 8],
                    pps[:],
                )

                # ---- lif convert on ScE (uint16 -> f32), before V copies ----
                lif = smpool.tile([C, NCAND], f32, tag="lif")
                nc.scalar.copy(lif[:], li_s[:])

                # ---- V^T fp32 (PE after S; ScE 1024-wide copies) ----
                vt_s = vpool.tile([C, NJ], f32, tag="VT")
                for e in range(8):
                    for half in range(2):
                        v_ps = ps_w.tile([C, 1024], f32, tag="w")
                        for n in range(2):
                            col = half * 1024 + n * 512
                            nc.tensor.matmul(
                                v_ps[:, n * 512:(n + 1) * 512],
                                wv_s[:, e * C:(e + 1) * C],
                                c16h[:, col:col + 512],
                            )
                        nc.scalar.copy(
                            vt_s[:, e * NC + half * 1024: e * NC + (half + 1) * 1024],
                            v_ps[:],
                        )

                # ---- exact top-12 marking on cand ----
                t8a = smpool.tile([C, 8], f32, tag="t8a")
                nc.vector.max(t8a[:], cand_s[:])
                c2 = smpool.tile([C, NCAND], f32, tag="c2")
                nc.vector.match_replace(c2[:], t8a[:], cand_s[:], NEG)
                t8b = smpool.tile([C, 8], f32, tag="t8b")
                nc.vector.max(t8b[:], c2[:])
                nc.vector.tensor_scalar(
                    t8b[:, 4:8], t8b[:, 4:8], 0.0, 1e30, Alu.mult, Alu.add
                )
                rr = smpool.tile([C, NCAND], f32, tag="rr")
                nc.vector.match_replace(rr[:], t8b[:], c2[:], NEG)

                # ---- pack global_idx*1024 + q10(value); mask; extract ----
                gfl = smpool.tile([C, NCAND], f32, tag="gfl")
                nc.vector.scalar_tensor_tensor(
                    gfl[:], lif[:], 1024.0, choff_s[:], Alu.mult, Alu.add
                )
                q10 = smpool.tile([C, NCAND], f32, tag="q10")
                nc.vector.tensor_scalar(
                    q10[:], cand_s[:], 4.0, 128.0, Alu.add, Alu.mult
                )
                pmsum = smpool.tile([C, NCAND], f32, tag="pmsum")
                nc.vector.scalar_tensor_tensor(
                    pmsum[:], gfl[:], 1.0, q10[:], Alu.mult, Alu.add
                )
                pm = smpool.tile([C, NCAND], f32, tag="pm")
                nc.vector.scalar_tensor_tensor(
                    pm[:], rr[:], -1e29, pmsum[:], Alu.is_le, Alu.mult
                )

                pw = smpool.tile([C, 16], f32, tag="pw")
                nc.vector.max(pw[:, 0:8], pm[:])
                pm2 = smpool.tile([C, NCAND], f32, tag="pm2")
                nc.vector.match_replace(pm2[:], pw[:, 0:8], pm[:], 0.0)
                nc.vector.max(pw[:, 8:16], pm2[:])

                # ---- decode winners: gidx + value -> weights ----
                gidxf = smpool.tile([C, 16], f32, tag="gidxf")
                nc.vector.tensor_scalar(
                    gidxf[:], pw[:], 1.0 / 1024.0, -0.5, Alu.mult, Alu.add
                )
                nc.vector.tensor_scalar(
                    gidxf[:], gidxf[:], MAGIC, MAGIC, Alu.add, Alu.subtract
                )
                # winner index first so the gather starts ASAP
                gp_i = smpool.tile([C, TK], dt.int16, tag="gpi")
                nc.vector.tensor_copy(gp_i[:], gidxf[:, 0:TK])
                g_s = smpool.tile([C, TK * NQ], f32, tag="G")
                nc.gpsimd.ap_gather(
                    g_s[:], vt_s[:], gp_i[:],
                    channels=C, num_elems=NJ, d=1, num_idxs=TK * NQ,
                )

                vv = smpool.tile([C, 16], f32, tag="vv")
                nc.vector.scalar_tensor_tensor(
                    vv[:], gidxf[:], -1024.0, pw[:], Alu.mult, Alu.add
                )
                nc.vector.tensor_scalar(
                    vv[:], vv[:], 1.0 / 128.0, -4.0, Alu.mult, Alu.add
                )
                expv = smpool.tile([C, 16], f32, tag="expv")
                nc.scalar.activation(
                    expv[:], vv[:], mybir.ActivationFunctionType.Exp
                )
                wgt = smpool.tile([C, 16], f32, tag="wgt")
                den = smpool.tile([C, 1], f32, tag="den")
                nc.vector.scalar_tensor_tensor(
                    wgt[:], pw[:], 0.5, expv[:], Alu.is_ge, Alu.mult,
                    accum_out=den[:],
                )
                rden = smpool.tile([C, 1], f32, tag="rden")
                nc.vector.reciprocal(rden[:], den[:])
                wn = smpool.tile([C, 16], f32, tag="wn")
                nc.vector.tensor_scalar(wn[:], wgt[:], rden[:], None, Alu.mult)

                # ---- weights -> [(h,d), (i,q)] via headrep matmul ----
                wnb = (
                    wn[:, 0:TK].rearrange("p (i o) -> p i o", o=1)
                    .to_broadcast([C, TK, 16])
                )
                wsc = smpool.tile([C, TK * NQ], f32, tag="wsc")
                nc.vector.tensor_mul(
                    wsc[:].rearrange("p (i s) -> p i s", s=16),
                    wnb,
                    m192_s[:].rearrange("p (i s) -> p i s", s=16),
                )
                wb_ps = ps_w.tile([C, 1024], f32, tag="w")
                nc.tensor.matmul(wb_ps[:, 0:TK * NQ], hrep_s[:], wsc[:])

                gw = smpool.tile([C, TK * NQ], f32, tag="gw")
                nc.vector.tensor_mul(gw[:], g_s[:], wb_ps[:, 0:TK * NQ])
                # reduce over i, keep q: write PV^T into (q,b) cols
                nc.vector.tensor_reduce(
                    pvt4_s[:, b::BPC],
                    gw[:].rearrange("p (i q) -> p q i", q=NQ),
                    mybir.AxisListType.X,
                    Alu.add,
                )

            # ---- final projection (Wp folded into Wjw on host) ----
            # out[b, c_out] = sum_q pvt4[:, (q,b)]^T @ wjwp[:, (q,c_out)]
            # 4 interleaved accumulation chains to hide LDWEIGHTS/accum latency
            o1_ps = ps_w.tile([C, 1024], f32, tag="w")
            for q in range(NQ):
                nc.tensor.matmul(
                    o1_ps[0:BPC, 0:C],
                    pvt4_s[:, q * BPC:(q + 1) * BPC],
                    wjwp_s[:, q * C:(q + 1) * C],
                    start=(q == 0),
                    stop=(q == NQ - 1),
                )
            o4_s = smpool.tile([BPC, C], f32, tag="o4")
            nc.vector.tensor_add(o4_s[:], o1_ps[0:BPC, 0:C], xp4_s[:])
            nc.sync.dma_start(out_d[:], o4_s[:])

    nc.compile()
    return nc


def _host_prep(inputs):
    x = np.asarray(inputs["x"], dtype=np.float32)              # [32, 1, 128]
    complement = np.asarray(inputs["complement"], np.float32)  # [32, 2047, 128]
    Wq = np.asarray(inputs["Wq"], np.float32)
    Wkv = np.asarray(inputs["Wkv"], np.float32)
    Wjw = np.asarray(inputs["Wjw"], np.float32)
    Wp = np.asarray(inputs["Wp"], np.float32)
    bp = np.asarray(inputs["bp"], np.float32)

    wv = np.empty((C, 8 * C), np.float32)
    for e in range(8):
        wv[:, e * C:(e + 1) * C] = Wkv[:, e * 256 + 128: e * 256 + 256]
    wv = wv.astype(np.float16)
    # host A: A[b][c, (e, h, q)] = 0.25 * sum_hd Wk_e[c, (h,hd)] q[b, q, h, hd]
    kproj = Wkv.reshape(C, 8, 2, H, HD)[:, :, 0]          # [c, e, h, hd]
    qtb = (x.reshape(B, C) @ Wq).reshape(B, NQ, H, HD)    # [b, q, h, hd]
    a_all = 0.25 * np.einsum('cehd,bqhd->bcehq', kproj, qtb, optimize=True)
    a_all = a_all.reshape(B, C, 8 * C).astype(np.float16)
    # fold Wp: out = pv @ (Wjw @ Wp) + (x @ Wp + bp)
    Wjw2 = (Wjw.astype(np.float64) @ Wp.astype(np.float64)).astype(np.float32)
    wjwp = (
        Wjw2.reshape(H, NQ, HD, C).transpose(1, 0, 2, 3).reshape(NQ, C, C)
        .transpose(1, 0, 2).reshape(C, NQ * C)
    )
    hrep = np.kron(np.eye(H, dtype=np.float32), np.ones((HD, HD), np.float32))
    choffrow = ((np.arange(NCAND) // 8) * (CHUNK * 1024)).astype(np.float32)
    choff = np.tile(choffrow.reshape(1, NCAND), (C, 1))
    # m192[p, (i, s16)] = [s16 == p % 16], i in [0, 12)
    s_idx = np.tile(np.arange(16).reshape(1, 1, 16), (C, TK, 1))
    p_idx = (np.arange(C) % NQ).reshape(C, 1, 1)
    m192 = (s_idx == p_idx).astype(np.float32).reshape(C, TK * NQ)

    shared = dict(
        wv=np.ascontiguousarray(wv),
        wjwp=np.ascontiguousarray(wjwp),
        hrep=np.ascontiguousarray(hrep),
        choff=np.ascontiguousarray(choff),
        m192=np.ascontiguousarray(m192),
    )

    in_maps = []
    for core in range(CORES):
        bs = range(core * BPC, (core + 1) * BPC)
        comp = np.stack(
            [
                np.concatenate([x[b].reshape(1, C), complement[b]], axis=0)
                for b in bs
            ]
        ).astype(np.float32)
        compT = comp.transpose(0, 2, 1)          # [BPC, C, NC]
        comphT = compT.astype(np.float16)
        xb = x[list(bs)].reshape(BPC, C)
        xp4 = np.ascontiguousarray((xb @ Wp + bp).astype(np.float32))
        m = dict(shared)
        m["comphT"] = np.ascontiguousarray(comphT)
        m["a16h4"] = np.ascontiguousarray(a_all[list(bs)])
        m["xp4"] = xp4
        in_maps.append(m)
    return in_maps


def kernel(**inputs):
    from concourse.bass_utils import run_bass_kernel_spmd

    if "prog" not in _prog_cache:
        _prog_cache["prog"] = _build_program()
    nc = _prog_cache["prog"]

    in_maps = _host_prep(inputs)
    res = run_bass_kernel_spmd(nc, in_maps, core_ids=list(range(CORES)))
    out = np.empty((B, 1, C), np.float32)
    for core in range(CORES):
        o = res.results[core]["out"]
        for i in range(BPC):
            out[core * BPC + i, 0, :] = o[i]
    return out


if __name__ == "__main__":
    d = np.load("/root/problem/inputs_cache.npz")
    inputs = {k: d[k] for k in d.files}
    got = kernel(**inputs)
    print("kernel output:", got.shape, got.dtype, np.abs(got).max())


# revision 30
# speedup vs baseline: 1.0221x; 1.0030x over previous
"""Trainium2 Bass kernel for nn_MultiHeadCrossAttention (B=32, Nc=2048, H=8, topk=12).

kernel(**inputs) takes FULL inputs, returns FULL output [32, 1, 128].
Batch is sharded 4-per-core across 8 NeuronCores (data parallel, no collectives).

Per-batch device algorithm (rows=(h,q) 128 wide, j = e*2048+nc in [0,16384)):
  hoisted for all 4 batches: qbd (block-diag 0.25-scaled Q), A_e fp16
  S_chunk[row, 1024] = Ah.T @ Ch single fp16 term, directly in PSUM
  per-chunk top8 (DVE max8) + max_index read straight from PSUM
  VT_e[hd,nc] = Wv_e.T @ Ch -> VT [128,16384] fp32 (ScE 1024-wide copies)
  exact global top-12 marking via max8/match_replace rounds on cand
  pack (global_idx*1024 + quantized_value), extract winners via max8
  weights = exp(value)/sum
  G = ap_gather(VT, winner idx, d=1 fp32)
  PV^T[(h,d),q] = sum w*G  (headrep matmul broadcasts weights)
  out = (PV flat @ WjwP) + x;  out = out @ Wp + bp
"""

import sys
import numpy as np

for p in ("/opt/trn_rl_repo",):
    if p not in sys.path:
        sys.path.insert(0, p)

import ml_dtypes

B, CORES, BPC = 32, 8, 4
H, HD, NQ, TK, C, NC = 8, 16, 16, 12, 128, 2048
NJ = 8 * NC            # 16384
CHUNK = 1024
NCH = NJ // CHUNK      # 16
NCAND = NCH * 8        # 128
NEG = -1e30
MAGIC = 12582912.0     # 2**23 + 2**22: add/sub rounds fp32 to nearest int

_prog_cache = {}


def _build_program():
    import concourse.bass as bass
    import concourse.mybir as mybir
    import concourse.tile as tile
    from concourse import bacc
    from concourse import library_config

    dt = mybir.dt
    Alu = mybir.AluOpType
    f32, f16, bf16 = dt.float32, dt.float16, dt.bfloat16
    nc = bacc.Bacc("TRN2", target_bir_lowering=False)

    comphT_d = nc.dram_tensor("comphT", [BPC, C, NC], f16, kind="ExternalInput")
    a16h4_d = nc.dram_tensor("a16h4", [BPC, C, 8 * C], f16, kind="ExternalInput")
    wv_d = nc.dram_tensor("wv", [C, 8 * C], f16, kind="ExternalInput")
    wjwp_d = nc.dram_tensor("wjwp", [C, NQ * C], f32, kind="ExternalInput")
    xp4_d = nc.dram_tensor("xp4", [BPC, C], f32, kind="ExternalInput")
    hrep_d = nc.dram_tensor("hrep", [C, C], f32, kind="ExternalInput")
    choff_d = nc.dram_tensor("choff", [C, NCAND], f32, kind="ExternalInput")
    m192_d = nc.dram_tensor("m192", [C, 192], f32, kind="ExternalInput")
    out_d = nc.dram_tensor("out", [BPC, C], f32, kind="ExternalOutput")

    with tile.TileContext(nc) as tc:
        nc.gpsimd.load_library(library_config.ap_gather)
        with (
            tc.tile_pool(name="weights", bufs=1) as wpool,
            tc.tile_pool(name="compt", bufs=3) as ctpool,
            tc.tile_pool(name="bigV", bufs=1) as vpool,
            tc.tile_pool(name="small", bufs=3) as smpool,
            tc.tile_pool(name="ps_s", bufs=2, space="PSUM") as ps_s,
            tc.tile_pool(name="ps_w", bufs=2, space="PSUM") as ps_w,
        ):
            # ---- critical-path DMAs first: A for b0, comp for b0 ----
            a16h_l = []
            for b in range(BPC):
                t = wpool.tile([C, 8 * C], f16, tag=f"a16h{b}")
                a16h_l.append(t)
            nc.sync.dma_start(a16h_l[0][:], a16h4_d[0])
            c16h_0 = ctpool.tile([C, NC], f16, tag="c16h")
            nc.scalar.dma_start(c16h_0[:], comphT_d[0])
            for b in range(1, BPC):
                nc.sync.dma_start(a16h_l[b][:], a16h4_d[b])
            wv_s = wpool.tile([C, 8 * C], f16)
            nc.sync.dma_start(wv_s[:], wv_d[:])
            wjwp_s = wpool.tile([C, NQ * C], f32)
            nc.sync.dma_start(wjwp_s[:], wjwp_d[:])
            xp4_s = wpool.tile([BPC, C], f32)
            nc.sync.dma_start(xp4_s[:], xp4_d[:])
            hrep_s = wpool.tile([C, C], f32)
            nc.sync.dma_start(hrep_s[:], hrep_d[:])
            choff_s = wpool.tile([C, NCAND], f32)
            nc.sync.dma_start(choff_s[:], choff_d[:])
            m192_s = wpool.tile([C, 192], f32)
            nc.sync.dma_start(m192_s[:], m192_d[:])

            pvt4_s = wpool.tile([C, NQ * BPC], f32)   # [(h,d), (q,b)]

            for b in range(BPC):
                a16h = a16h_l[b]
                if b == 0:
                    c16h = c16h_0
                else:
                    c16h = ctpool.tile([C, NC], f16, tag="c16h")
                    nc.sync.dma_start(c16h[:], comphT_d[b])

                # ---- S chunks in PSUM; scans software-pipelined so each
                # find_index consumes the PREVIOUS chunk (breaks RAW stalls) ----
                cand_s = smpool.tile([C, NCAND], f32, tag="cand")
                li_s = smpool.tile([C, NCAND], dt.uint16, tag="li")
                prev = None
                for ch in range(NCH):
                    e, half = ch // 2, ch % 2
                    ah = a16h[:, e * C:(e + 1) * C]
                    s_ps = ps_s.tile([C, CHUNK], f32, tag="s")
                    for n in range(2):
                        col = half * 1024 + n * 512
                        nc.tensor.matmul(
                            s_ps[:, n * 512:(n + 1) * 512],
                            ah, c16h[:, col:col + 512],
                        )
                    nc.vector.max(cand_s[:, ch * 8:(ch + 1) * 8], s_ps[:])
                    if prev is not None:
                        pch, pps = prev
                        nc.vector.max_index(
                            li_s[:, pch * 8:(pch + 1) * 8],
                            cand_s[:, pch * 8:(pch + 1) * 8],
                            pps[:],
                        )
                    prev = (ch, s_ps)
                pch, pps = prev
                nc.vector.max_index(
                    li_s[:, pch * 8:(pch + 1) * 8],
                    cand_s[:, pch * 8:(pch + 1) * 8],
                    pps[:],
                )

                # ---- lif convert on ScE (uint16 -> f32), before V copies ----
                lif = smpool.tile([C, NCAND], f32, tag="lif")
                nc.scalar.copy(lif[:], li_s[:])

                # ---- V^T fp32 (PE after S; ScE 1024-wide copies) ----
                vt_s = vpool.tile([C, NJ], f32, tag="VT")
                for e in range(8):
                    for half in range(2):
                        v_ps = ps_w.tile([C, 1024], f32, tag="w")
                        for n in range(2):
                            col = half * 1024 + n * 512
                            nc.tensor.matmul(
                                v_ps[:, n * 512:(n + 1) * 512],
                                wv_s[:, e * C:(e + 1) * C],
                                c16h[:, col:col + 512],
                            )
                        nc.scalar.copy(
                            vt_s[:, e * NC + half * 1024: e * NC + (half + 1) * 1024],
                            v_ps[:],
                        )

                # ---- exact top-12 marking on cand ----
                t8a = smpool.tile([C, 8], f32, tag="t8a")
                nc.vector.max(t8a[:], cand_s[:])
                c2 = smpool.tile([C, NCAND], f32, tag="c2")
                nc.vector.match_replace(c2[:], t8a[:], cand_s[:], NEG)
                t8b = smpool.tile([C, 8], f32, tag="t8b")
                nc.vector.max(t8b[:], c2[:])
                nc.vector.tensor_scalar(
                    t8b[:, 4:8], t8b[:, 4:8], 0.0, 1e30, Alu.mult, Alu.add
                )
                rr = smpool.tile([C, NCAND], f32, tag="rr")
                nc.vector.match_replace(rr[:], t8b[:], c2[:], NEG)

                # ---- pack global_idx*1024 + q10(value); mask; extract ----
                gfl = smpool.tile([C, NCAND], f32, tag="gfl")
                nc.vector.scalar_tensor_tensor(
                    gfl[:], lif[:], 1024.0, choff_s[:], Alu.mult, Alu.add
                )
                q10 = smpool.tile([C, NCAND], f32, tag="q10")
                nc.vector.tensor_scalar(
                    q10[:], cand_s[:], 4.0, 128.0, Alu.add, Alu.mult
                )
                pmsum = smpool.tile([C, NCAND], f32, tag="pmsum")
                nc.vector.scalar_tensor_tensor(
                    pmsum[:], gfl[:], 1.0, q10[:], Alu.mult, Alu.add
                )
                pm = smpool.tile([C, NCAND], f32, tag="pm")
                nc.vector.scalar_tensor_tensor(
                    pm[:], rr[:], -1e29, pmsum[:], Alu.is_le, Alu.mult
                )

                pw = smpool.tile([C, 16], f32, tag="pw")
                nc.vector.max(pw[:, 0:8], pm[:])
                pm2 = smpool.tile([C, NCAND], f32, tag="pm2")
                nc.vector.match_replace(pm2[:], pw[:, 0:8], pm[:], 0.0)
                nc.vector.max(pw[:, 8:16], pm2[:])

                # ---- decode winners: gidx + value -> weights ----
                gidxf = smpool.tile([C, 16], f32, tag="gidxf")
                nc.vector.tensor_scalar(
                    gidxf[:], pw[:], 1.0 / 1024.0, -0.5, Alu.mult, Alu.add
                )
                nc.vector.tensor_scalar(
                    gidxf[:], gidxf[:], MAGIC, MAGIC, Alu.add, Alu.subtract
                )
                # winner index first so the gather starts ASAP
                gp_i = smpool.tile([C, TK], dt.int16, tag="gpi")
                nc.vector.tensor_copy(gp_i[:], gidxf[:, 0:TK])
                g_s = smpool.tile([C, TK * NQ], f32, tag="G")
                nc.gpsimd.ap_gather(
                    g_s[:], vt_s[:], gp_i[:],
                    channels=C, num_elems=NJ, d=1, num_idxs=TK * NQ,
                )

                vv = smpool.tile([C, 16], f32, tag="vv")
                nc.vector.scalar_tensor_tensor(
                    vv[:], gidxf[:], -1024.0, pw[:], Alu.mult, Alu.add
                )
                nc.vector.tensor_scalar(
                    vv[:], vv[:], 1.0 / 128.0, -4.0, Alu.mult, Alu.add
                )
                expv = smpool.tile([C, 16], f32, tag="expv")
                nc.scalar.activation(
                    expv[:], vv[:], mybir.ActivationFunctionType.Exp
                )
                wgt = smpool.tile([C, 16], f32, tag="wgt")
                den = smpool.tile([C, 1], f32, tag="den")
                nc.vector.scalar_tensor_tensor(
                    wgt[:], pw[:], 0.5, expv[:], Alu.is_ge, Alu.mult,
                    accum_out=den[:],
                )
                rden = smpool.tile([C, 1], f32, tag="rden")
                nc.vector.reciprocal(rden[:], den[:])
                wn = smpool.tile([C, 16], f32, tag="wn")
                nc.vector.tensor_scalar(wn[:], wgt[:], rden[:], None, Alu.mult)

                # ---- weights -> [(h,d), (i,q)] via headrep matmul ----
                wnb = (
                    wn[:, 0:TK].rearrange("p (i o) -> p i o", o=1)
                    .to_broadcast([C, TK, 16])
                )
                wsc = smpool.tile([C, TK * NQ], f32, tag="wsc")
                nc.vector.tensor_mul(
                    wsc[:].rearrange("p (i s) -> p i s", s=16),
                    wnb,
                    m192_s[:].rearrange("p (i s) -> p i s", s=16),
                )
                wb_ps = ps_w.tile([C, 1024], f32, tag="w")
                nc.tensor.matmul(wb_ps[:, 0:TK * NQ], hrep_s[:], wsc[:])

                gw = smpool.tile([C, TK * NQ], f32, tag="gw")
                nc.vector.tensor_mul(gw[:], g_s[:], wb_ps[:, 0:TK * NQ])
                # reduce over i, keep q: write PV^T into (q,b) cols
                nc.vector.tensor_reduce(
                    pvt4_s[:, b::BPC],
                    gw[:].rearrange("p (i q) -> p q i", q=NQ),
                    mybir.AxisListType.X,
                    Alu.add,
                )

            # ---- final projection (Wp folded into Wjw on host) ----
            # out[b, c_out] = sum_q pvt4[:, (q,b)]^T @ wjwp[:, (q,c_out)]
            # 4 interleaved accumulation chains to hide LDWEIGHTS/accum latency
            o1_ps = ps_w.tile([C, 1024], f32, tag="w")
            for q in range(NQ):
                nc.tensor.matmul(
                    o1_ps[0:BPC, 0:C],
                    pvt4_s[:, q * BPC:(q + 1) * BPC],
                    wjwp_s[:, q * C:(q + 1) * C],
                    start=(q == 0),
                    stop=(q == NQ - 1),
                )
            o4_s = smpool.tile([BPC, C], f32, tag="o4")
            nc.vector.tensor_add(o4_s[:], o1_ps[0:BPC, 0:C], xp4_s[:])
            nc.sync.dma_start(out_d[:], o4_s[:])

    nc.compile()
    return nc


def _host_prep(inputs):
    x = np.asarray(inputs["x"], dtype=np.float32)              # [32, 1, 128]
    complement = np.asarray(inputs["complement"], np.float32)  # [32, 2047, 128]
    Wq = np.asarray(inputs["Wq"], np.float32)
    Wkv = np.asarray(inputs["Wkv"], np.float32)
    Wjw = np.asarray(inputs["Wjw"], np.float32)
    Wp = np.asarray(inputs["Wp"], np.float32)
    bp = np.asarray(inputs["bp"], np.float32)

    wv = np.empty((C, 8 * C), np.float32)
    for e in range(8):
        wv[:, e * C:(e + 1) * C] = Wkv[:, e * 256 + 128: e * 256 + 256]
    wv = wv.astype(np.float16)
    # host A: A[b][c, (e, h, q)] = 0.25 * sum_hd Wk_e[c, (h,hd)] q[b, q, h, hd]
    kproj = Wkv.reshape(C, 8, 2, H, HD)[:, :, 0]          # [c, e, h, hd]
    qtb = (x.reshape(B, C) @ Wq).reshape(B, NQ, H, HD)    # [b, q, h, hd]
    a_all = 0.25 * np.einsum('cehd,bqhd->bcehq', kproj, qtb, optimize=True)
    a_all = a_all.reshape(B, C, 8 * C).astype(np.float16)
    # fold Wp: out = pv @ (Wjw @ Wp) + (x @ Wp + bp)
    Wjw2 = (Wjw.astype(np.float64) @ Wp.astype(np.float64)).astype(np.float32)
    wjwp = (
        Wjw2.reshape(H, NQ, HD, C).transpose(1, 0, 2, 3).reshape(NQ, C, C)
        .transpose(1, 0, 2).reshape(C, NQ * C)
    )
    hrep = np.kron(np.eye(H, dtype=np.float32), np.ones((HD, HD), np.float32))
    choffrow = ((np.arange(NCAND) // 8) * (CHUNK * 1024)).astype(np.float32)
    choff = np.tile(choffrow.reshape(1, NCAND), (C, 1))
    # m192[p, (i, s16)] = [s16 == p % 16], i in [0, 12)
    s_idx = np.tile(np.arange(16).reshape(1, 1, 16), (C, TK, 1))
    p_idx = (np.arange(C) % NQ).reshape(C, 1, 1)
    m192 = (s_idx == p_idx).astype(np.float32).reshape(C, TK * NQ)

    shared = dict(
        wv=np.ascontiguousarray(wv),
        wjwp=np.ascontiguousarray(wjwp),
        hrep=np.ascontiguousarray(hrep),
        choff=np.ascontiguousarray(choff),
        m192=np.ascontiguousarray(m192),
    )

    in_maps = []
    for core in range(CORES):
        bs = range(core * BPC, (core + 1) * BPC)
        comp = np.stack(
            [
                np.concatenate([x[b].reshape(1, C), complement[b]], axis=0)
                for b in bs
            ]
        ).astype(np.float32)
        compT = comp.transpose(0, 2, 1)          # [BPC, C, NC]
        comphT = compT.astype(np.float16)
        xb = x[list(bs)].reshape(BPC, C)
        xp4 = np.ascontiguousarray((xb @ Wp + bp).astype(np.float32))
        m = dict(shared)
        m["comphT"] = np.ascontiguousarray(comphT)
        m["a16h4"] = np.ascontiguousarray(a_all[list(bs)])
        m["xp4"] = xp4
        in_maps.append(m)
    return in_maps


def kernel(**inputs):
    from concourse.bass_utils import run_bass_kernel_spmd

    if "prog" not in _prog_cache:
        _prog_cache["prog"] = _build_program()
    nc = _prog_cache["prog"]

    in_maps = _host_prep(inputs)
    res = run_bass_kernel_spmd(nc, in_maps, core_ids=list(range(CORES)))
    out = np.empty((B, 1, C), np.float32)
    for core in range(CORES):
        o = res.results[core]["out"]
        for i in range(BPC):
            out[core * BPC + i, 0, :] = o[i]
    return out


if __name__ == "__main__":
    d = np.load("/root/problem/inputs_cache.npz")
    inputs = {k: d[k] for k in d.files}
    got = kernel(**inputs)
    print("kernel output:", got.shape, got.dtype, np.abs(got).max())


# revision 31
# speedup vs baseline: 1.0256x; 1.0034x over previous
"""Trainium2 Bass kernel for nn_MultiHeadCrossAttention (B=32, Nc=2048, H=8, topk=12).

kernel(**inputs) takes FULL inputs, returns FULL output [32, 1, 128].
Batch is sharded 4-per-core across 8 NeuronCores (data parallel, no collectives).

Per-batch device algorithm (rows=(h,q) 128 wide, j = e*2048+nc in [0,16384)):
  hoisted for all 4 batches: qbd (block-diag 0.25-scaled Q), A_e fp16
  S_chunk[row, 1024] = Ah.T @ Ch single fp16 term, directly in PSUM
  per-chunk top8 (DVE max8) + max_index read straight from PSUM
  VT_e[hd,nc] = Wv_e.T @ Ch -> VT [128,16384] fp32 (ScE 1024-wide copies)
  exact global top-12 marking via max8/match_replace rounds on cand
  pack (global_idx*1024 + quantized_value), extract winners via max8
  weights = exp(value)/sum
  G = ap_gather(VT, winner idx, d=1 fp32)
  PV^T[(h,d),q] = sum w*G  (headrep matmul broadcasts weights)
  out = (PV flat @ WjwP) + x;  out = out @ Wp + bp
"""

import sys
import numpy as np

for p in ("/opt/trn_rl_repo",):
    if p not in sys.path:
        sys.path.insert(0, p)

import ml_dtypes

B, CORES, BPC = 32, 8, 4
H, HD, NQ, TK, C, NC = 8, 16, 16, 12, 128, 2048
NJ = 8 * NC            # 16384
CHUNK = 1024
NCH = NJ // CHUNK      # 16
NCAND = NCH * 8        # 128
NEG = -1e30
MAGIC = 12582912.0     # 2**23 + 2**22: add/sub rounds fp32 to nearest int

_prog_cache = {}


def _build_program():
    import concourse.bass as bass
    import concourse.mybir as mybir
    import concourse.tile as tile
    from concourse import bacc
    from concourse import library_config

    dt = mybir.dt
    Alu = mybir.AluOpType
    f32, f16, bf16 = dt.float32, dt.float16, dt.bfloat16
    nc = bacc.Bacc("TRN2", target_bir_lowering=False)

    comphT_d = nc.dram_tensor("comphT", [BPC, C, NC], f16, kind="ExternalInput")
    a16h4_d = nc.dram_tensor("a16h4", [BPC, C, 8 * C], f16, kind="ExternalInput")
    wv_d = nc.dram_tensor("wv", [C, 8 * C], f16, kind="ExternalInput")
    wjwp_d = nc.dram_tensor("wjwp", [C, NQ * C], f32, kind="ExternalInput")
    xp4_d = nc.dram_tensor("xp4", [BPC, C], f32, kind="ExternalInput")
    hrep_d = nc.dram_tensor("hrep", [C, C], f32, kind="ExternalInput")
    choff_d = nc.dram_tensor("choff", [C, NCAND], f32, kind="ExternalInput")
    m192_d = nc.dram_tensor("m192", [C, 192], f32, kind="ExternalInput")
    out_d = nc.dram_tensor("out", [BPC, C], f32, kind="ExternalOutput")

    with tile.TileContext(nc) as tc:
        nc.gpsimd.load_library(library_config.ap_gather)
        with (
            tc.tile_pool(name="weights", bufs=1) as wpool,
            tc.tile_pool(name="compt", bufs=3) as ctpool,
            tc.tile_pool(name="bigV", bufs=1) as vpool,
            tc.tile_pool(name="small", bufs=3) as smpool,
            tc.tile_pool(name="ps_s", bufs=2, space="PSUM") as ps_s,
            tc.tile_pool(name="ps_w", bufs=2, space="PSUM") as ps_w,
        ):
            # ---- critical-path DMAs first: A for b0, comp for b0 ----
            a16h_l = []
            for b in range(BPC):
                t = wpool.tile([C, 8 * C], f16, tag=f"a16h{b}")
                a16h_l.append(t)
            nc.sync.dma_start(a16h_l[0][:], a16h4_d[0])
            c16h_0 = ctpool.tile([C, NC], f16, tag="c16h")
            nc.scalar.dma_start(c16h_0[:], comphT_d[0])
            for b in range(1, BPC):
                nc.sync.dma_start(a16h_l[b][:], a16h4_d[b])
            wv_s = wpool.tile([C, 8 * C], f16)
            nc.sync.dma_start(wv_s[:], wv_d[:])
            wjwp_s = wpool.tile([C, NQ * C], f32)
            nc.sync.dma_start(wjwp_s[:], wjwp_d[:])
            xp4_s = wpool.tile([BPC, C], f32)
            nc.sync.dma_start(xp4_s[:], xp4_d[:])
            hrep_s = wpool.tile([C, C], f32)
            nc.sync.dma_start(hrep_s[:], hrep_d[:])
            choff_s = wpool.tile([C, NCAND], f32)
            nc.sync.dma_start(choff_s[:], choff_d[:])
            m192_s = wpool.tile([C, 192], f32)
            nc.sync.dma_start(m192_s[:], m192_d[:])

            pvt4_s = wpool.tile([C, NQ * BPC], f32)   # [(h,d), (q,b)]

            for b in range(BPC):
                a16h = a16h_l[b]
                if b == 0:
                    c16h = c16h_0
                else:
                    c16h = ctpool.tile([C, NC], f16, tag="c16h")
                    nc.sync.dma_start(c16h[:], comphT_d[b])

                # ---- S chunks in PSUM; scans software-pipelined so each
                # find_index consumes the PREVIOUS chunk (breaks RAW stalls) ----
                cand_s = smpool.tile([C, NCAND], f32, tag="cand")
                li_s = smpool.tile([C, NCAND], dt.uint16, tag="li")
                prev = None
                for ch in range(NCH):
                    e, half = ch // 2, ch % 2
                    ah = a16h[:, e * C:(e + 1) * C]
                    s_ps = ps_s.tile([C, CHUNK], f32, tag="s")
                    for n in range(2):
                        col = half * 1024 + n * 512
                        nc.tensor.matmul(
                            s_ps[:, n * 512:(n + 1) * 512],
                            ah, c16h[:, col:col + 512],
                        )
                    nc.vector.max(cand_s[:, ch * 8:(ch + 1) * 8], s_ps[:])
                    if prev is not None:
                        pch, pps = prev
                        nc.vector.max_index(
                            li_s[:, pch * 8:(pch + 1) * 8],
                            cand_s[:, pch * 8:(pch + 1) * 8],
                            pps[:],
                        )
                    prev = (ch, s_ps)
                pch, pps = prev
                nc.vector.max_index(
                    li_s[:, pch * 8:(pch + 1) * 8],
                    cand_s[:, pch * 8:(pch + 1) * 8],
                    pps[:],
                )

                # ---- lif convert on ScE (uint16 -> f32), before V copies ----
                lif = smpool.tile([C, NCAND], f32, tag="lif")
                nc.scalar.copy(lif[:], li_s[:])

                # ---- V^T fp32 (PE after S; ScE 1024-wide copies) ----
                vt_s = vpool.tile([C, NJ], f32, tag="VT")
                for e in range(8):
                    for half in range(2):
                        v_ps = ps_w.tile([C, 1024], f32, tag="w")
                        for n in range(2):
                            col = half * 1024 + n * 512
                            nc.tensor.matmul(
                                v_ps[:, n * 512:(n + 1) * 512],
                                wv_s[:, e * C:(e + 1) * C],
                                c16h[:, col:col + 512],
                            )
                        nc.scalar.copy(
                            vt_s[:, e * NC + half * 1024: e * NC + (half + 1) * 1024],
                            v_ps[:],
                        )

                # ---- exact top-12 marking on cand (pack ops slotted
                # into the chain's producer->consumer gaps) ----
                t8a = smpool.tile([C, 8], f32, tag="t8a")
                nc.vector.max(t8a[:], cand_s[:])
                q10 = smpool.tile([C, NCAND], f32, tag="q10")
                nc.vector.tensor_scalar(
                    q10[:], cand_s[:], 4.0, 128.0, Alu.add, Alu.mult
                )
                c2 = smpool.tile([C, NCAND], f32, tag="c2")
                nc.vector.match_replace(c2[:], t8a[:], cand_s[:], NEG)
                gfl = smpool.tile([C, NCAND], f32, tag="gfl")
                nc.vector.scalar_tensor_tensor(
                    gfl[:], lif[:], 1024.0, choff_s[:], Alu.mult, Alu.add
                )
                t8b = smpool.tile([C, 8], f32, tag="t8b")
                nc.vector.max(t8b[:], c2[:])
                nc.vector.tensor_scalar(
                    t8b[:, 4:8], t8b[:, 4:8], 0.0, 1e30, Alu.mult, Alu.add
                )
                rr = smpool.tile([C, NCAND], f32, tag="rr")
                nc.vector.match_replace(rr[:], t8b[:], c2[:], NEG)

                # ---- pack global_idx*1024 + q10(value); mask; extract ----
                pmsum = smpool.tile([C, NCAND], f32, tag="pmsum")
                nc.vector.scalar_tensor_tensor(
                    pmsum[:], gfl[:], 1.0, q10[:], Alu.mult, Alu.add
                )
                pm = smpool.tile([C, NCAND], f32, tag="pm")
                nc.vector.scalar_tensor_tensor(
                    pm[:], rr[:], -1e29, pmsum[:], Alu.is_le, Alu.mult
                )

                pw = smpool.tile([C, 16], f32, tag="pw")
                nc.vector.max(pw[:, 0:8], pm[:])
                pm2 = smpool.tile([C, NCAND], f32, tag="pm2")
                nc.vector.match_replace(pm2[:], pw[:, 0:8], pm[:], 0.0)
                nc.vector.max(pw[:, 8:16], pm2[:])

                # ---- decode winners: gidx + value -> weights ----
                gidxf = smpool.tile([C, 16], f32, tag="gidxf")
                nc.vector.tensor_scalar(
                    gidxf[:], pw[:], 1.0 / 1024.0, -0.5, Alu.mult, Alu.add
                )
                nc.vector.tensor_scalar(
                    gidxf[:], gidxf[:], MAGIC, MAGIC, Alu.add, Alu.subtract
                )
                # winner index first so the gather starts ASAP
                gp_i = smpool.tile([C, TK], dt.int16, tag="gpi")
                nc.vector.tensor_copy(gp_i[:], gidxf[:, 0:TK])
                g_s = smpool.tile([C, TK * NQ], f32, tag="G")
                nc.gpsimd.ap_gather(
                    g_s[:], vt_s[:], gp_i[:],
                    channels=C, num_elems=NJ, d=1, num_idxs=TK * NQ,
                )

                vv = smpool.tile([C, 16], f32, tag="vv")
                nc.vector.scalar_tensor_tensor(
                    vv[:], gidxf[:], -1024.0, pw[:], Alu.mult, Alu.add
                )
                nc.vector.tensor_scalar(
                    vv[:], vv[:], 1.0 / 128.0, -4.0, Alu.mult, Alu.add
                )
                expv = smpool.tile([C, 16], f32, tag="expv")
                nc.scalar.activation(
                    expv[:], vv[:], mybir.ActivationFunctionType.Exp
                )
                wgt = smpool.tile([C, 16], f32, tag="wgt")
                den = smpool.tile([C, 1], f32, tag="den")
                nc.vector.scalar_tensor_tensor(
                    wgt[:], pw[:], 0.5, expv[:], Alu.is_ge, Alu.mult,
                    accum_out=den[:],
                )
                rden = smpool.tile([C, 1], f32, tag="rden")
                nc.vector.reciprocal(rden[:], den[:])
                wn = smpool.tile([C, 16], f32, tag="wn")
                nc.vector.tensor_scalar(wn[:], wgt[:], rden[:], None, Alu.mult)

                # ---- weights -> [(h,d), (i,q)] via headrep matmul ----
                wnb = (
                    wn[:, 0:TK].rearrange("p (i o) -> p i o", o=1)
                    .to_broadcast([C, TK, 16])
                )
                wsc = smpool.tile([C, TK * NQ], f32, tag="wsc")
                nc.vector.tensor_mul(
                    wsc[:].rearrange("p (i s) -> p i s", s=16),
                    wnb,
                    m192_s[:].rearrange("p (i s) -> p i s", s=16),
                )
                wb_ps = ps_w.tile([C, 1024], f32, tag="w")
                nc.tensor.matmul(wb_ps[:, 0:TK * NQ], hrep_s[:], wsc[:])

                gw = smpool.tile([C, TK * NQ], f32, tag="gw")
                nc.vector.tensor_mul(gw[:], g_s[:], wb_ps[:, 0:TK * NQ])
                # reduce over i, keep q: write PV^T into (q,b) cols
                nc.vector.tensor_reduce(
                    pvt4_s[:, b::BPC],
                    gw[:].rearrange("p (i q) -> p q i", q=NQ),
                    mybir.AxisListType.X,
                    Alu.add,
                )

            # ---- final projection (Wp folded into Wjw on host) ----
            # out[b, c_out] = sum_q pvt4[:, (q,b)]^T @ wjwp[:, (q,c_out)]
            # 4 interleaved accumulation chains to hide LDWEIGHTS/accum latency
            o1_ps = ps_w.tile([C, 1024], f32, tag="w")
            for q in range(NQ):
                nc.tensor.matmul(
                    o1_ps[0:BPC, 0:C],
                    pvt4_s[:, q * BPC:(q + 1) * BPC],
                    wjwp_s[:, q * C:(q + 1) * C],
                    start=(q == 0),
                    stop=(q == NQ - 1),
                )
            o4_s = smpool.tile([BPC, C], f32, tag="o4")
            nc.vector.tensor_add(o4_s[:], o1_ps[0:BPC, 0:C], xp4_s[:])
            nc.sync.dma_start(out_d[:], o4_s[:])

    nc.compile()
    return nc


def _host_prep(inputs):
    x = np.asarray(inputs["x"], dtype=np.float32)              # [32, 1, 128]
    complement = np.asarray(inputs["complement"], np.float32)  # [32, 2047, 128]
    Wq = np.asarray(inputs["Wq"], np.float32)
    Wkv = np.asarray(inputs["Wkv"], np.float32)
    Wjw = np.asarray(inputs["Wjw"], np.float32)
    Wp = np.asarray(inputs["Wp"], np.float32)
    bp = np.asarray(inputs["bp"], np.float32)

    wv = np.empty((C, 8 * C), np.float32)
    for e in range(8):
        wv[:, e * C:(e + 1) * C] = Wkv[:, e * 256 + 128: e * 256 + 256]
    wv = wv.astype(np.float16)
    # host A: A[b][c, (e, h, q)] = 0.25 * sum_hd Wk_e[c, (h,hd)] q[b, q, h, hd]
    kproj = Wkv.reshape(C, 8, 2, H, HD)[:, :, 0]          # [c, e, h, hd]
    qtb = (x.reshape(B, C) @ Wq).reshape(B, NQ, H, HD)    # [b, q, h, hd]
    a_all = 0.25 * np.einsum('cehd,bqhd->bcehq', kproj, qtb, optimize=True)
    a_all = a_all.reshape(B, C, 8 * C).astype(np.float16)
    # fold Wp: out = pv @ (Wjw @ Wp) + (x @ Wp + bp)
    Wjw2 = (Wjw.astype(np.float64) @ Wp.astype(np.float64)).astype(np.float32)
    wjwp = (
        Wjw2.reshape(H, NQ, HD, C).transpose(1, 0, 2, 3).reshape(NQ, C, C)
        .transpose(1, 0, 2).reshape(C, NQ * C)
    )
    hrep = np.kron(np.eye(H, dtype=np.float32), np.ones((HD, HD), np.float32))
    choffrow = ((np.arange(NCAND) // 8) * (CHUNK * 1024)).astype(np.float32)
    choff = np.tile(choffrow.reshape(1, NCAND), (C, 1))
    # m192[p, (i, s16)] = [s16 == p % 16], i in [0, 12)
    s_idx = np.tile(np.arange(16).reshape(1, 1, 16), (C, TK, 1))
    p_idx = (np.arange(C) % NQ).reshape(C, 1, 1)
    m192 = (s_idx == p_idx).astype(np.float32).reshape(C, TK * NQ)

    shared = dict(
        wv=np.ascontiguousarray(wv),
        wjwp=np.ascontiguousarray(wjwp),
        hrep=np.ascontiguousarray(hrep),
        choff=np.ascontiguousarray(choff),
        m192=np.ascontiguousarray(m192),
    )

    in_maps = []
    for core in range(CORES):
        bs = range(core * BPC, (core + 1) * BPC)
        comp = np.stack(
            [
                np.concatenate([x[b].reshape(1, C), complement[b]], axis=0)
                for b in bs
            ]
        ).astype(np.float32)
        compT = comp.transpose(0, 2, 1)          # [BPC, C, NC]
        comphT = compT.astype(np.float16)
        xb = x[list(bs)].reshape(BPC, C)
        xp4 = np.ascontiguousarray((xb @ Wp + bp).astype(np.float32))
        m = dict(shared)
        m["comphT"] = np.ascontiguousarray(comphT)
        m["a16h4"] = np.ascontiguousarray(a_all[list(bs)])
        m["xp4"] = xp4
        in_maps.append(m)
    return in_maps


def kernel(**inputs):
    from concourse.bass_utils import run_bass_kernel_spmd

    if "prog" not in _prog_cache:
        _prog_cache["prog"] = _build_program()
    nc = _prog_cache["prog"]

    in_maps = _host_prep(inputs)
    res = run_bass_kernel_spmd(nc, in_maps, core_ids=list(range(CORES)))
    out = np.empty((B, 1, C), np.float32)
    for core in range(CORES):
        o = res.results[core]["out"]
        for i in range(BPC):
            out[core * BPC + i, 0, :] = o[i]
    return out


if __name__ == "__main__":
    d = np.load("/root/problem/inputs_cache.npz")
    inputs = {k: d[k] for k in d.files}
    got = kernel(**inputs)
    print("kernel output:", got.shape, got.dtype, np.abs(got).max())


# revision 32
# speedup vs baseline: 1.0356x; 1.0097x over previous
"""Trainium2 Bass kernel for nn_MultiHeadCrossAttention (B=32, Nc=2048, H=8, topk=12).

kernel(**inputs) takes FULL inputs, returns FULL output [32, 1, 128].
Batch is sharded 4-per-core across 8 NeuronCores (data parallel, no collectives).

Per-batch device algorithm (rows=(h,q) 128 wide, j = e*2048+nc in [0,16384)):
  hoisted for all 4 batches: qbd (block-diag 0.25-scaled Q), A_e fp16
  S_chunk[row, 1024] = Ah.T @ Ch single fp16 term, directly in PSUM
  per-chunk top8 (DVE max8) + max_index read straight from PSUM
  VT_e[hd,nc] = Wv_e.T @ Ch -> VT [128,16384] fp32 (ScE 1024-wide copies)
  exact global top-12 marking via max8/match_replace rounds on cand
  pack (global_idx*1024 + quantized_value), extract winners via max8
  weights = exp(value)/sum
  G = ap_gather(VT, winner idx, d=1 fp32)
  PV^T[(h,d),q] = sum w*G  (headrep matmul broadcasts weights)
  out = (PV flat @ WjwP) + x;  out = out @ Wp + bp
"""

import sys
import numpy as np

for p in ("/opt/trn_rl_repo",):
    if p not in sys.path:
        sys.path.insert(0, p)

import ml_dtypes

B, CORES, BPC = 32, 8, 4
H, HD, NQ, TK, C, NC = 8, 16, 16, 12, 128, 2048
NJ = 8 * NC            # 16384
CHUNK = 1024
NCH = NJ // CHUNK      # 16
NCAND = NCH * 8        # 128
NEG = -1e30
MAGIC = 12582912.0     # 2**23 + 2**22: add/sub rounds fp32 to nearest int

_prog_cache = {}


def _build_program():
    import concourse.bass as bass
    import concourse.mybir as mybir
    import concourse.tile as tile
    from concourse import bacc
    from concourse import library_config

    dt = mybir.dt
    Alu = mybir.AluOpType
    f32, f16, bf16 = dt.float32, dt.float16, dt.bfloat16
    nc = bacc.Bacc("TRN2", target_bir_lowering=False)

    comphT_d = nc.dram_tensor("comphT", [BPC, C, NC], f16, kind="ExternalInput")
    a16h4_d = nc.dram_tensor("a16h4", [BPC, C, 8 * C], f16, kind="ExternalInput")
    wv_d = nc.dram_tensor("wv", [C, 8 * C], f16, kind="ExternalInput")
    wjwp_d = nc.dram_tensor("wjwp", [C, NQ * C], f32, kind="ExternalInput")
    xp4_d = nc.dram_tensor("xp4", [BPC, C], f32, kind="ExternalInput")
    hrep_d = nc.dram_tensor("hrep", [C, C], f32, kind="ExternalInput")
    choff_d = nc.dram_tensor("choff", [C, NCAND], f32, kind="ExternalInput")
    m192_d = nc.dram_tensor("m192", [C, 192], f32, kind="ExternalInput")
    out_d = nc.dram_tensor("out", [BPC, C], f32, kind="ExternalOutput")

    with tile.TileContext(nc) as tc:
        nc.gpsimd.load_library(library_config.ap_gather)
        with (
            tc.tile_pool(name="weights", bufs=1) as wpool,
            tc.tile_pool(name="compt", bufs=3) as ctpool,
            tc.tile_pool(name="bigV", bufs=1) as vpool,
            tc.tile_pool(name="small", bufs=3) as smpool,
            tc.tile_pool(name="ps_s", bufs=2, space="PSUM") as ps_s,
            tc.tile_pool(name="ps_w", bufs=2, space="PSUM") as ps_w,
        ):
            # ---- critical-path DMAs first: A for b0, comp for b0 ----
            a16h_l = []
            for b in range(BPC):
                t = wpool.tile([C, 8 * C], f16, tag=f"a16h{b}")
                a16h_l.append(t)
            nc.sync.dma_start(a16h_l[0][:], a16h4_d[0])
            c16h_0 = ctpool.tile([C, NC], f16, tag="c16h")
            nc.scalar.dma_start(c16h_0[:], comphT_d[0])
            for b in range(1, BPC):
                nc.sync.dma_start(a16h_l[b][:], a16h4_d[b])
            wv_s = wpool.tile([C, 8 * C], f16)
            nc.sync.dma_start(wv_s[:], wv_d[:])
            wjwp_s = wpool.tile([C, NQ * C], f32)
            nc.sync.dma_start(wjwp_s[:], wjwp_d[:])
            xp4_s = wpool.tile([BPC, C], f32)
            nc.sync.dma_start(xp4_s[:], xp4_d[:])
            hrep_s = wpool.tile([C, C], f32)
            nc.sync.dma_start(hrep_s[:], hrep_d[:])
            choff_s = wpool.tile([C, NCAND], f32)
            nc.sync.dma_start(choff_s[:], choff_d[:])
            m192_s = wpool.tile([C, 192], f32)
            nc.sync.dma_start(m192_s[:], m192_d[:])

            pvt4_s = wpool.tile([C, NQ * BPC], f32)   # [(h,d), (q,b)]

            for b in range(BPC):
                a16h = a16h_l[b]
                if b == 0:
                    c16h = c16h_0
                else:
                    c16h = ctpool.tile([C, NC], f16, tag="c16h")
                    nc.sync.dma_start(c16h[:], comphT_d[b])

                # ---- S chunks in PSUM; scans software-pipelined so each
                # find_index consumes the PREVIOUS chunk (breaks RAW stalls) ----
                cand_s = smpool.tile([C, NCAND], f32, tag="cand")
                li_s = smpool.tile([C, NCAND], dt.uint16, tag="li")
                prev = None
                for ch in range(NCH):
                    e, half = ch // 2, ch % 2
                    ah = a16h[:, e * C:(e + 1) * C]
                    s_ps = ps_s.tile([C, CHUNK], f32, tag="s")
                    for n in range(2):
                        col = half * 1024 + n * 512
                        nc.tensor.matmul(
                            s_ps[:, n * 512:(n + 1) * 512],
                            ah, c16h[:, col:col + 512],
                        )
                    nc.vector.max(cand_s[:, ch * 8:(ch + 1) * 8], s_ps[:])
                    if prev is not None:
                        pch, pps = prev
                        nc.vector.max_index(
                            li_s[:, pch * 8:(pch + 1) * 8],
                            cand_s[:, pch * 8:(pch + 1) * 8],
                            pps[:],
                        )
                    prev = (ch, s_ps)
                pch, pps = prev
                nc.vector.max_index(
                    li_s[:, pch * 8:(pch + 1) * 8],
                    cand_s[:, pch * 8:(pch + 1) * 8],
                    pps[:],
                )

                # ---- lif convert on ScE (uint16 -> f32), before V copies ----
                lif = smpool.tile([C, NCAND], f32, tag="lif")
                nc.scalar.copy(lif[:], li_s[:])

                # ---- V^T fp32 (PE after S; ScE 1024-wide copies) ----
                vt_s = vpool.tile([C, NJ], f32, tag="VT")
                for e in range(8):
                    for half in range(2):
                        v_ps = ps_w.tile([C, 1024], f32, tag="w")
                        for n in range(2):
                            col = half * 1024 + n * 512
                            nc.tensor.matmul(
                                v_ps[:, n * 512:(n + 1) * 512],
                                wv_s[:, e * C:(e + 1) * C],
                                c16h[:, col:col + 512],
                            )
                        nc.scalar.copy(
                            vt_s[:, e * NC + half * 1024: e * NC + (half + 1) * 1024],
                            v_ps[:],
                        )

                # ---- exact top-12 marking on cand (pack ops slotted
                # into the chain's producer->consumer gaps) ----
                t8a = smpool.tile([C, 8], f32, tag="t8a")
                nc.vector.max(t8a[:], cand_s[:])
                q10 = smpool.tile([C, NCAND], f32, tag="q10")
                nc.vector.tensor_scalar(
                    q10[:], cand_s[:], 4.0, 128.0, Alu.add, Alu.mult
                )
                c2 = smpool.tile([C, NCAND], f32, tag="c2")
                nc.vector.match_replace(c2[:], t8a[:], cand_s[:], NEG)
                gfl = smpool.tile([C, NCAND], f32, tag="gfl")
                nc.vector.scalar_tensor_tensor(
                    gfl[:], lif[:], 1024.0, choff_s[:], Alu.mult, Alu.add
                )
                t8b = smpool.tile([C, 8], f32, tag="t8b")
                nc.vector.max(t8b[:], c2[:])
                nc.vector.tensor_scalar(
                    t8b[:, 4:8], t8b[:, 4:8], 0.0, 1e30, Alu.mult, Alu.add
                )
                rr = smpool.tile([C, NCAND], f32, tag="rr")
                nc.vector.match_replace(rr[:], t8b[:], c2[:], NEG)

                # ---- pack global_idx*1024 + q10(value); mask; extract ----
                pmsum = smpool.tile([C, NCAND], f32, tag="pmsum")
                nc.vector.scalar_tensor_tensor(
                    pmsum[:], gfl[:], 1.0, q10[:], Alu.mult, Alu.add
                )
                pm = smpool.tile([C, NCAND], f32, tag="pm")
                nc.vector.scalar_tensor_tensor(
                    pm[:], rr[:], -1e29, pmsum[:], Alu.is_le, Alu.mult
                )

                pw = smpool.tile([C, 16], f32, tag="pw")
                nc.vector.max(pw[:, 0:8], pm[:])
                pm2 = smpool.tile([C, NCAND], f32, tag="pm2")
                nc.vector.match_replace(pm2[:], pw[:, 0:8], pm[:], 0.0)
                nc.vector.max(pw[:, 8:16], pm2[:])

                # ---- decode winners: gidx + value -> weights ----
                gidxf = smpool.tile([C, 16], f32, tag="gidxf")
                nc.vector.tensor_scalar(
                    gidxf[:], pw[:], 1.0 / 1024.0, -0.5, Alu.mult, Alu.add
                )
                nc.vector.tensor_scalar(
                    gidxf[:], gidxf[:], MAGIC, MAGIC, Alu.add, Alu.subtract
                )
                # winner index first so the gather starts ASAP
                gp_i = smpool.tile([C, TK], dt.int16, tag="gpi")
                nc.vector.tensor_copy(gp_i[:], gidxf[:, 0:TK])
                g_s = smpool.tile([C, TK * NQ], f32, tag="G")
                nc.gpsimd.ap_gather(
                    g_s[:], vt_s[:], gp_i[:],
                    channels=C, num_elems=NJ, d=1, num_idxs=TK * NQ,
                )

                vv = smpool.tile([C, 16], f32, tag="vv")
                nc.vector.scalar_tensor_tensor(
                    vv[:], gidxf[:], -1024.0, pw[:], Alu.mult, Alu.add
                )
                nc.vector.tensor_scalar(
                    vv[:], vv[:], 1.0 / 128.0, -4.0, Alu.mult, Alu.add
                )
                expv = smpool.tile([C, 16], f32, tag="expv")
                nc.scalar.activation(
                    expv[:], vv[:], mybir.ActivationFunctionType.Exp
                )
                wgt = smpool.tile([C, 16], f32, tag="wgt")
                den = smpool.tile([C, 1], f32, tag="den")
                nc.vector.scalar_tensor_tensor(
                    wgt[:], pw[:], 0.5, expv[:], Alu.is_ge, Alu.mult,
                    accum_out=den[:],
                )
                rden = smpool.tile([C, 1], f32, tag="rden")
                nc.vector.reciprocal(rden[:], den[:])

                # ---- weights -> [(h,d), (i,q)] via headrep matmul ----
                wgb = (
                    wgt[:, 0:TK].rearrange("p (i o) -> p i o", o=1)
                    .to_broadcast([C, TK, 16])
                )
                wsc = smpool.tile([C, TK * NQ], f32, tag="wsc")
                nc.vector.scalar_tensor_tensor(
                    wsc[:].rearrange("p (i s) -> p i s", s=16),
                    wgb, rden[:],
                    m192_s[:].rearrange("p (i s) -> p i s", s=16),
                    Alu.mult, Alu.mult,
                )
                wb_ps = ps_w.tile([C, 1024], f32, tag="w")
                nc.tensor.matmul(wb_ps[:, 0:TK * NQ], hrep_s[:], wsc[:])

                gw = smpool.tile([C, TK * NQ], f32, tag="gw")
                nc.vector.tensor_mul(gw[:], g_s[:], wb_ps[:, 0:TK * NQ])
                # reduce over i, keep q: write PV^T into (q,b) cols
                nc.vector.tensor_reduce(
                    pvt4_s[:, b::BPC],
                    gw[:].rearrange("p (i q) -> p q i", q=NQ),
                    mybir.AxisListType.X,
                    Alu.add,
                )

            # ---- final projection (Wp folded into Wjw on host) ----
            # out[b, c_out] = sum_q pvt4[:, (q,b)]^T @ wjwp[:, (q,c_out)]
            # 4 interleaved accumulation chains to hide LDWEIGHTS/accum latency
            o1_ps = ps_w.tile([C, 1024], f32, tag="w")
            for q in range(NQ):
                nc.tensor.matmul(
                    o1_ps[0:BPC, 0:C],
                    pvt4_s[:, q * BPC:(q + 1) * BPC],
                    wjwp_s[:, q * C:(q + 1) * C],
                    start=(q == 0),
                    stop=(q == NQ - 1),
                )
            o4_s = smpool.tile([BPC, C], f32, tag="o4")
            nc.vector.tensor_add(o4_s[:], o1_ps[0:BPC, 0:C], xp4_s[:])
            nc.sync.dma_start(out_d[:], o4_s[:])

    nc.compile()
    return nc


def _host_prep(inputs):
    x = np.asarray(inputs["x"], dtype=np.float32)              # [32, 1, 128]
    complement = np.asarray(inputs["complement"], np.float32)  # [32, 2047, 128]
    Wq = np.asarray(inputs["Wq"], np.float32)
    Wkv = np.asarray(inputs["Wkv"], np.float32)
    Wjw = np.asarray(inputs["Wjw"], np.float32)
    Wp = np.asarray(inputs["Wp"], np.float32)
    bp = np.asarray(inputs["bp"], np.float32)

    wv = np.empty((C, 8 * C), np.float32)
    for e in range(8):
        wv[:, e * C:(e + 1) * C] = Wkv[:, e * 256 + 128: e * 256 + 256]
    wv = wv.astype(np.float16)
    # host A: A[b][c, (e, h, q)] = 0.25 * sum_hd Wk_e[c, (h,hd)] q[b, q, h, hd]
    kproj = Wkv.reshape(C, 8, 2, H, HD)[:, :, 0]          # [c, e, h, hd]
    qtb = (x.reshape(B, C) @ Wq).reshape(B, NQ, H, HD)    # [b, q, h, hd]
    a_all = 0.25 * np.einsum('cehd,bqhd->bcehq', kproj, qtb, optimize=True)
    a_all = a_all.reshape(B, C, 8 * C).astype(np.float16)
    # fold Wp: out = pv @ (Wjw @ Wp) + (x @ Wp + bp)
    Wjw2 = (Wjw.astype(np.float64) @ Wp.astype(np.float64)).astype(np.float32)
    wjwp = (
        Wjw2.reshape(H, NQ, HD, C).transpose(1, 0, 2, 3).reshape(NQ, C, C)
        .transpose(1, 0, 2).reshape(C, NQ * C)
    )
    hrep = np.kron(np.eye(H, dtype=np.float32), np.ones((HD, HD), np.float32))
    choffrow = ((np.arange(NCAND) // 8) * (CHUNK * 1024)).astype(np.float32)
    choff = np.tile(choffrow.reshape(1, NCAND), (C, 1))
    # m192[p, (i, s16)] = [s16 == p % 16], i in [0, 12)
    s_idx = np.tile(np.arange(16).reshape(1, 1, 16), (C, TK, 1))
    p_idx = (np.arange(C) % NQ).reshape(C, 1, 1)
    m192 = (s_idx == p_idx).astype(np.float32).reshape(C, TK * NQ)

    shared = dict(
        wv=np.ascontiguousarray(wv),
        wjwp=np.ascontiguousarray(wjwp),
        hrep=np.ascontiguousarray(hrep),
        choff=np.ascontiguousarray(choff),
        m192=np.ascontiguousarray(m192),
    )

    in_maps = []
    for core in range(CORES):
        bs = range(core * BPC, (core + 1) * BPC)
        comp = np.stack(
            [
                np.concatenate([x[b].reshape(1, C), complement[b]], axis=0)
                for b in bs
            ]
        ).astype(np.float32)
        compT = comp.transpose(0, 2, 1)          # [BPC, C, NC]
        comphT = compT.astype(np.float16)
        xb = x[list(bs)].reshape(BPC, C)
        xp4 = np.ascontiguousarray((xb @ Wp + bp).astype(np.float32))
        m = dict(shared)
        m["comphT"] = np.ascontiguousarray(comphT)
        m["a16h4"] = np.ascontiguousarray(a_all[list(bs)])
        m["xp4"] = xp4
        in_maps.append(m)
    return in_maps


def kernel(**inputs):
    from concourse.bass_utils import run_bass_kernel_spmd

    if "prog" not in _prog_cache:
        _prog_cache["prog"] = _build_program()
    nc = _prog_cache["prog"]

    in_maps = _host_prep(inputs)
    res = run_bass_kernel_spmd(nc, in_maps, core_ids=list(range(CORES)))
    out = np.empty((B, 1, C), np.float32)
    for core in range(CORES):
        o = res.results[core]["out"]
        for i in range(BPC):
            out[core * BPC + i, 0, :] = o[i]
    return out


if __name__ == "__main__":
    d = np.load("/root/problem/inputs_cache.npz")
    inputs = {k: d[k] for k in d.files}
    got = kernel(**inputs)
    print("kernel output:", got.shape, got.dtype, np.abs(got).max())
